# revision 15
# baseline (speedup 1.0000x reference)
"""Trainium2 Bass kernel for nn_Neural_time_50337016709696 (embedding_lookup + RFF).

Computation (reference):
    Uvec[b] = concat_m U[m, b_i_n[b, m]]            # [B, 96] gather
    x[b]    = concat(Uvec[b], t[b])                 # [B, 97]
    proj    = x @ W_freq                            # [B, 256]
    y       = (sin(proj)@w1 + cos(proj)@w2)/16 + b  # [B, 1]

Device strategy (data-parallel over batch, 8 cores, replicated tables):
  * fp16 gather table [3*nvec, 32] (mode offset folded into the index on
    host). Indirect DMAs (one index column = 128 descriptors each) pull
    all U rows into the per-batch-row layout X[128p, 96*g+32*m+e].
  * PE transposes X tiles (fp16) into feature-major Xt[99, 1024]; row 96
    is t[b] (DMA'd), rows 97/98 are ones (memset).
  * pc = x @ (W/2pi) on PE (PSUM fp32); W row 97 = 1536, row 98 = phi/2pi
    (w1*sin(p)+w2*cos(p) = A*sin(p+phi), A/phi precomputed on host), so
    pc = (proj+phi)/2pi + 1536 in [1527, 1546] in the fp16 integer-grid
    binade [1024, 2048).
  * u = fp16(pc) = round(pc) exactly (one cast-copy per chunk, split
    between DVE and ACT to balance engine load).
  * DVE tensor_tensor: z = u - pc in [-0.5, 0.5] (fp16 out).
  * ACT Sin: s = sin(-2pi*z) = sin(proj+phi).
  * y = A_c.T @ s_c accumulated on PE (col-tiled over 4 PSUM positions);
    ACT Identity adds b_out; strided-partition DMA writes y (host
    de-interleaves).
"""

import math

import numpy as np

import concourse.bass as bass
import concourse.mybir as mybir
import concourse.tile as tile
from concourse import bacc
from concourse.bass import IndirectOffsetOnAxis
from concourse.masks import make_identity

P = 128
R = 32
NMOD = 3
NVEC = 500000
NFF = 256
IN_DIM = NMOD * R + 1  # 97
IN2 = 99               # + 1536-row + phase-row
N_CORES = 8
B = 131072
B_LOC = B // N_CORES  # 16384

F32 = mybir.dt.float32
F16 = mybir.dt.float16
I32 = mybir.dt.int32

SIN = mybir.ActivationFunctionType.Sin
IDENT = mybir.ActivationFunctionType.Identity
TWO_PI = float(2.0 * math.pi)
POFF = 1536.0          # places pc in the fp16 integer-grid binade

# index columns per indirect-DMA instruction. HW-probed: the indirect DMA
# consumes exactly ONE index per partition per instruction (multi-column
# offsets gather garbage beyond the first column), so this must stay 1.
# The 384 gathers' ~1.09us/instruction serial SWDGE descriptor-gen is the
# kernel's critical path (~418us/rep measured in isolation); all compute
# engines (PE/DVE/ACT, each under ~95us) hide beneath it, so ACT is kept
# Sin-only (no table-set switches) and everything else runs on DVE.
GCOLS = 1

# SWDGE queue fan-out: the ucode supports up to 4 qPoolDynamic{i} queue
# contexts; spreading the indirect DMAs round-robin across them lets the
# Q7 descriptor generation overlap across contexts.
N_SWDGE_QUEUES = 1


def indirect_gather_on_queue(gp, out, in_, offset_ap, queue_name):
    """Clone of BassGpSimd.indirect_dma_start (gather direction) with a
    selectable qPoolDynamic{i} queue name."""
    out_ap = gp.lower_ap_dma(out, for_indirect_dma=True)
    in_ap = gp.lower_ap_dma(in_, for_indirect_dma=True)
    assert len(in_ap) == 1 and len(out_ap) == 1
    off_ap = gp.lower_ap_dma(offset_ap)
    assert len(off_ap) == 1
    in_ap.append(off_ap[0])
    ap_shape = in_.shape
    coef = 1
    for i in range(1, len(ap_shape)):
        coef *= ap_shape[i]
    in_ap[0].dynamic_ap_info = mybir.DynamicAccessPatternInfo(
        c=0,
        actual_ap=out.ap,
        indirect_dim_max_index=ap_shape[0],
        offset_expr=[
            mybir.DynamicAccessPatternOffsetExpr(
                coef=coef,
                aff_expr=mybir.DynamicAccessPatternOffsetExprAffExpr(
                    kind="IndirectArgId", arg_id=1),
            )
        ],
    )
    return gp.add_instruction(
        mybir.InstDMACopy(
            name=gp.bass.get_next_instruction_name(),
            queue=queue_name,
            mode="Copy",
            ins=in_ap,
            outs=out_ap,
            oob_is_err=True,
            cce_op=mybir.AluOpType.bypass,
        )
    )


def emit_kernel(tc, u_ap, idx_ap, t_ap, w_ap, amp_ap, b_ap, y_ap,
                b_loc=B_LOC, n_chunks=4, reps=1, gcols=GCOLS):
    """Emit the per-core program. All *_ap are DRAM APs:
      u_ap    [NMOD*nvec, R] f16   (gather table, mode-major)
      idx_ap  [128, 3*G] i32       idx[p, 3g+m] = b_i[g*128+p, m] + m*nvec
      t_ap    [3, b_loc] f16       row 0 = b_t_n; rows 1,2 = 1.0
      w_ap    [99, 256] f16        rows 0..96 W_freq/2pi; 97: 1536; 98: phi/2pi
      amp_ap  [128, 64] f16        col 32c+j = A[128c+p]; A=hypot(w1,w2)
      b_ap    [128, 1] f32         b_out replicated
      y_ap    [4, b_loc//4] f32    y_ap[q, 512*S+n] = y[2048*S + 512*q + n]
    """
    nc = tc.nc
    G = b_loc // P                  # groups of 128 batches
    GPB = 8                         # groups per compute block (1024 batches)
    assert G % GPB == 0
    NB = G // GPB
    assert NB % 2 == 0
    NS = NB // 2                    # superblocks of 2048 batches
    assert G % n_chunks == 0
    gpc = G // n_chunks             # groups per gather chunk
    assert gpc % GPB == 0
    bpc = gpc // GPB                # blocks per chunk

    from contextlib import ExitStack
    with ExitStack() as ctx:
        const_pool = ctx.enter_context(tc.tile_pool(name="const", bufs=1))
        idx_pool = ctx.enter_context(tc.tile_pool(name="idx", bufs=2))
        x_pool = ctx.enter_context(tc.tile_pool(name="xdata", bufs=1))
        xt_pool = ctx.enter_context(tc.tile_pool(name="xt", bufs=2))
        u_pool = ctx.enter_context(tc.tile_pool(name="uround", bufs=2))
        z_pool = ctx.enter_context(tc.tile_pool(name="zfrac", bufs=2))
        s_pool = ctx.enter_context(tc.tile_pool(name="sins", bufs=2))
        y_pool = ctx.enter_context(tc.tile_pool(name="yout", bufs=2))
        pt_pool = ctx.enter_context(tc.tile_pool(name="ptr", bufs=2, space="PSUM"))
        pr_pool = ctx.enter_context(tc.tile_pool(name="prj", bufs=1, space="PSUM"))
        py_pool = ctx.enter_context(tc.tile_pool(name="pyy", bufs=2, space="PSUM"))

        # constants to SBUF (loaded once; weights stay resident)
        w_sb = const_pool.tile([IN2, NFF], F16, name="w_sb")
        amp_sb = const_pool.tile([P, 64], F16, name="amp_sb")
        b_sb = const_pool.tile([P, 1], F32, name="b_sb")
        ident = const_pool.tile([P, P], F16, name="ident")

        zf32 = const_pool.tile([P, 1024], F32, name="zf32")
        zf16 = const_pool.tile([P, 512], F16, name="zf16")
        bb = const_pool.tile([P, 512], F32, name="bb")
        nc.vector.memset(zf32[:, :], 0.0)
        nc.vector.memset(zf16[:, :], 0.0)
        nc.sync.dma_start(out=w_sb[:, :], in_=w_ap)
        nc.sync.dma_start(out=amp_sb[:, :], in_=amp_ap)
        nc.sync.dma_start(out=b_sb[:, :], in_=b_ap)
        nc.vector.tensor_scalar_add(
            out=bb[:, :], in0=zf32[:, 0:512], scalar1=b_sb[0:P, 0:1])
        make_identity(nc, ident[:, :])

        # gather chunks: X_q [128, gpc*96] f16; X[p, 96 g + 32 m + e]
        #   = U[idx[p, 3g+m], e].  One indirect DMA covers `gcols` index
        #   columns (descriptor k pairs idx[p, j] with X[p, 32j:32j+32]).
        def do_gathers(idx_sb, rep):
            x_tiles = []
            n_inst = 0
            for q in range(n_chunks):
                xq = x_pool.tile([P, 96 * gpc], F16, tag=f"xq{q}",
                                 name=f"xq{q}_r{rep}")
                cols = 3 * gpc
                for j0 in range(0, cols, gcols):
                    j1 = min(j0 + gcols, cols)
                    jj0 = 3 * gpc * q + j0
                    jj1 = jj0 + (j1 - j0)
                    nc.gpsimd.indirect_dma_start(
                        out=xq[:, 32 * j0:32 * j1],
                        out_offset=None,
                        in_=u_ap,
                        in_offset=IndirectOffsetOnAxis(
                            ap=idx_sb[:, jj0:jj1], axis=0),
                    )
                x_tiles.append(xq)
            return x_tiles

        def do_block(x_tiles, rep, i, py, s_pos):
            """block i = 1024 batches; y accumulated into psum tile `py`
            at col-tile positions {2*s_pos, 2*s_pos+1}."""
            q = i // bpc
            xq = x_tiles[q]
            base_g = i * GPB - q * gpc

            xt = xt_pool.tile([P, 1024], F16, tag="xt",
                              name=f"xt{i}_r{rep}")
            # rows 96 (t) and 97/98 (ones) in one DMA, off the Pool queue
            nc.sync.dma_start(out=xt[96:99, :],
                              in_=t_ap[0:3, 1024 * i:1024 * (i + 1)])
            for h in range(2):
                pt = pt_pool.tile([P, 512], F16, tag="pt",
                                  name=f"pt{i}_{h}_r{rep}")
                for gl in range(4):
                    g_in = base_g + h * 4 + gl
                    nc.tensor.transpose(
                        out=pt[0:96, 128 * gl:128 * (gl + 1)],
                        in_=xq[:, 96 * g_in:96 * (g_in + 1)],
                        identity=ident[:, :])
                nc.vector.tensor_tensor(
                    out=xt[0:96, 512 * h:512 * (h + 1)],
                    in0=pt[0:96, 0:512], in1=zf16[0:96, 0:512],
                    op=mybir.AluOpType.add)

            sss = []
            for c in range(2):
                pc = pr_pool.tile([P, 1024], F32, tag=f"proj{c}",
                                  name=f"proj{c}_{i}_r{rep}")
                for h2 in range(2):
                    nc.tensor.matmul(
                        out=pc[:, 512 * h2:512 * (h2 + 1)],
                        lhsT=w_sb[0:IN2, 128 * c:128 * (c + 1)],
                        rhs=xt[0:IN2, 512 * h2:512 * (h2 + 1)],
                        start=True, stop=True)
                # u = round(pc): fp16 cast; pc sits in [1024, 2048) where
                # the fp16 grid is exactly the integers
                uc = u_pool.tile([P, 1024], F16, tag=f"u{c}",
                                 name=f"u{c}_{i}_r{rep}")
                nc.vector.tensor_tensor(
                    out=uc[:, :], in0=pc[:, :], in1=zf32[:, :],
                    op=mybir.AluOpType.add)
                # z = u - pc in [-0.5, 0.5]
                zc = z_pool.tile([P, 1024], F16, tag=f"z{c}",
                                 name=f"z{c}_{i}_r{rep}")
                nc.vector.tensor_tensor(
                    out=zc[:, :], in0=uc[:, :], in1=pc[:, :],
                    op=mybir.AluOpType.subtract)
                # s = sin(-2pi*z) = sin(proj + phi)
                sc = s_pool.tile([P, 1024], F16, tag=f"s{c}",
                                 name=f"s{c}_{i}_r{rep}")
                nc.scalar.activation(out=sc[:, :], in_=zc[:, :], func=SIN,
                                     bias=0.0, scale=-TWO_PI)
                sss.append(sc)

            for h2 in range(2):
                pos = 2 * s_pos + h2
                for c in range(2):
                    nc.tensor.matmul(
                        out=py[32 * pos:32 * pos + 32, :],
                        lhsT=amp_sb[0:P, 32 * c:32 * (c + 1)],
                        rhs=sss[c][:, 512 * h2:512 * (h2 + 1)],
                        start=(c == 0), stop=(c == 1),
                        tile_position=(0, 32 * pos))

        for rep in range(reps):
            # per-inference inputs: index array (re-)loaded every rep
            idx_sb = idx_pool.tile([P, 3 * G], I32, tag="idx",
                                   name=f"idx_sb_r{rep}")
            nc.sync.dma_start(out=idx_sb[:, :], in_=idx_ap)
            x_tiles = do_gathers(idx_sb, rep)
            for S in range(NS):
                py = py_pool.tile([P, 512], F32, tag="py",
                                  name=f"py{S}_r{rep}")
                do_block(x_tiles, rep, 2 * S, py, 0)
                do_block(x_tiles, rep, 2 * S + 1, py, 1)
                # full-width copy + b_out; DMA picks rows {0,32,64,96}
                ys = y_pool.tile([P, 512], F32, tag="ystage",
                                 name=f"ys{S}_r{rep}")
                nc.vector.tensor_tensor(
                    out=ys[:, :], in0=py[:, :], in1=bb[:, :],
                    op=mybir.AluOpType.add)
                nc.sync.dma_start(out=y_ap[0:4, 512 * S:512 * (S + 1)],
                                  in_=ys[0:128:32, 0:512])


def build_program(b_loc=B_LOC, nvec=NVEC, n_chunks=4, n_cores=N_CORES,
                  reps=1, gcols=GCOLS):
    """Build the full Bass module (one SPMD program for all cores)."""
    G = b_loc // P
    nc = bacc.Bacc("TRN2", target_bir_lowering=False, debug=False,
                   num_devices=n_cores, name="rff_embed",
                   num_swdge_queues=N_SWDGE_QUEUES)
    u_d = nc.dram_tensor("u_tab", [NMOD * nvec, R], F16, kind="ExternalInput").ap()
    idx_d = nc.dram_tensor("idx", [P, 3 * G], I32, kind="ExternalInput").ap()
    t_d = nc.dram_tensor("tvec", [3, b_loc], F16, kind="ExternalInput").ap()
    w_d = nc.dram_tensor("wfreq", [IN2, NFF], F16, kind="ExternalInput").ap()
    amp_d = nc.dram_tensor("amp", [P, 64], F16, kind="ExternalInput").ap()
    b_d = nc.dram_tensor("bout", [P, 1], F32, kind="ExternalInput").ap()
    y_d = nc.dram_tensor("y", [4, b_loc // 4], F32, kind="ExternalOutput").ap()

    with tile.TileContext(nc) as tc:
        emit_kernel(tc, u_d, idx_d, t_d, w_d, amp_d, b_d, y_d,
                    b_loc=b_loc, n_chunks=n_chunks, reps=reps, gcols=gcols)
    nc.compile()
    return nc


def prep_shared(U, W_freq, w_out, b_out, nvec=NVEC):
    """Host prep of the replicated (weight) tensors."""
    u_arr = np.ascontiguousarray(
        np.asarray(U).reshape(NMOD * nvec, R).astype(np.float16))
    inv = 1.0 / np.sqrt(np.float64(NFF))
    w_out = np.asarray(w_out)
    w1 = w_out[:NFF, 0].astype(np.float64) * inv
    w2 = w_out[NFF:, 0].astype(np.float64) * inv
    amp = np.hypot(w1, w2)
    phi = np.arctan2(w2, w1)
    w_arr = np.empty((IN2, NFF), np.float64)
    w_arr[:IN_DIM] = np.asarray(W_freq, np.float64) / (2 * np.pi)
    w_arr[IN_DIM] = POFF
    w_arr[IN_DIM + 1] = phi / (2 * np.pi)
    w_arr = np.ascontiguousarray(w_arr.astype(np.float16))
    amp_arr = np.empty((P, 64), np.float64)
    amp_arr[:, 0:32] = amp[:P, None]
    amp_arr[:, 32:64] = amp[P:, None]
    amp_arr = np.ascontiguousarray(amp_arr.astype(np.float16))
    b_arr = np.full((P, 1), np.asarray(b_out).reshape(()), np.float32)
    return u_arr, w_arr, amp_arr, b_arr


def prep_core(b_i, b_t, nvec=NVEC):
    """Host prep of one core's sharded index / t tensors."""
    b_loc = b_i.shape[0]
    G = b_loc // P
    offs = (np.arange(NMOD, dtype=np.int64) * nvec)
    idx = (np.asarray(b_i, np.int64).reshape(G, P, NMOD) + offs[None, None, :])
    idx = np.ascontiguousarray(
        idx.transpose(1, 0, 2).reshape(P, 3 * G).astype(np.int32))
    t1 = np.ones((3, b_loc), np.float16)
    t1[0] = np.asarray(b_t, np.float32).reshape(b_loc).astype(np.float16)
    return idx, t1


def unscramble_y(y_d, b_loc):
    """y_d [4, b_loc//4] -> y [b_loc] natural order."""
    ns = b_loc // 2048
    return np.ascontiguousarray(
        y_d.reshape(4, ns, 512).transpose(1, 0, 2).reshape(b_loc))


_PROGRAM_CACHE = {}


def kernel(b_i_n, b_t_n, U, W_freq, w_out, b_out):
    from concourse.bass_utils import run_bass_kernel_spmd

    key = "full"
    if key not in _PROGRAM_CACHE:
        _PROGRAM_CACHE[key] = build_program()
    nc = _PROGRAM_CACHE[key]

    u_arr, w_arr, amp_arr, b_arr = prep_shared(U, W_freq, w_out, b_out)
    in_maps = []
    for k in range(N_CORES):
        sl = slice(k * B_LOC, (k + 1) * B_LOC)
        idx, t1 = prep_core(np.asarray(b_i_n)[sl], np.asarray(b_t_n)[sl])
        in_maps.append({"u_tab": u_arr, "idx": idx, "tvec": t1,
                        "wfreq": w_arr, "amp": amp_arr, "bout": b_arr})

    res = run_bass_kernel_spmd(nc, in_maps, core_ids=list(range(N_CORES)))
    y = np.concatenate([unscramble_y(r["y"], B_LOC) for r in res.results])
    return y.reshape(B, 1).astype(np.float32)



# revision 16
# speedup vs baseline: 1.2247x; 1.2247x over previous
"""Trainium2 Bass kernel for nn_Neural_time_50337016709696 (embedding_lookup + RFF).

Computation (reference):
    Uvec[b] = concat_m U[m, b_i_n[b, m]]            # [B, 96] gather
    x[b]    = concat(Uvec[b], t[b])                 # [B, 97]
    proj    = x @ W_freq                            # [B, 256]
    y       = (sin(proj)@w1 + cos(proj)@w2)/16 + b  # [B, 1]

Device strategy (data-parallel over batch, 8 cores, replicated tables):
  * fp16 gather table [3*nvec, 32] (mode offset folded into the index on
    host). Indirect DMAs (one index column = 128 descriptors each) pull
    all U rows into the per-batch-row layout X[128p, 96*g+32*m+e].
  * PE transposes X tiles (fp16) into feature-major Xt[99, 1024]; row 96
    is t[b] (DMA'd), rows 97/98 are ones (memset).
  * pc = x @ (W/2pi) on PE (PSUM fp32); W row 97 = 1536, row 98 = phi/2pi
    (w1*sin(p)+w2*cos(p) = A*sin(p+phi), A/phi precomputed on host), so
    pc = (proj+phi)/2pi + 1536 in [1527, 1546] in the fp16 integer-grid
    binade [1024, 2048).
  * u = fp16(pc) = round(pc) exactly (one cast-copy per chunk, split
    between DVE and ACT to balance engine load).
  * DVE tensor_tensor: z = u - pc in [-0.5, 0.5] (fp16 out).
  * ACT Sin: s = sin(-2pi*z) = sin(proj+phi).
  * y = A_c.T @ s_c accumulated on PE (col-tiled over 4 PSUM positions);
    ACT Identity adds b_out; strided-partition DMA writes y (host
    de-interleaves).
"""

import math

import numpy as np

import concourse.bass as bass
import concourse.mybir as mybir
import concourse.tile as tile
from concourse import bacc
from concourse.bass import IndirectOffsetOnAxis
from concourse.masks import make_identity

P = 128
R = 32
NMOD = 3
NVEC = 500000
NFF = 256
IN_DIM = NMOD * R + 1  # 97
IN2 = 99               # + 1536-row + phase-row
N_CORES = 8
B = 131072
B_LOC = B // N_CORES  # 16384

F32 = mybir.dt.float32
F16 = mybir.dt.float16
I32 = mybir.dt.int32

SIN = mybir.ActivationFunctionType.Sin
IDENT = mybir.ActivationFunctionType.Identity
TWO_PI = float(2.0 * math.pi)
POFF = 1536.0          # places pc in the fp16 integer-grid binade

# index columns per indirect-DMA instruction. HW-probed: the indirect DMA
# consumes exactly ONE index per partition per instruction (multi-column
# offsets gather garbage beyond the first column), so this must stay 1.
# The 384 gathers' ~1.09us/instruction serial SWDGE descriptor-gen is the
# kernel's critical path (~418us/rep measured in isolation); all compute
# engines (PE/DVE/ACT, each under ~95us) hide beneath it, so ACT is kept
# Sin-only (no table-set switches) and everything else runs on DVE.
GCOLS = 1

# SWDGE queue fan-out: the ucode supports up to 4 qPoolDynamic{i} queue
# contexts; spreading the indirect DMAs round-robin across them lets the
# Q7 descriptor generation overlap across contexts.
N_SWDGE_QUEUES = 1


def indirect_gather_on_queue(gp, out, in_, offset_ap, queue_name):
    """Clone of BassGpSimd.indirect_dma_start (gather direction) with a
    selectable qPoolDynamic{i} queue name."""
    out_ap = gp.lower_ap_dma(out, for_indirect_dma=True)
    in_ap = gp.lower_ap_dma(in_, for_indirect_dma=True)
    assert len(in_ap) == 1 and len(out_ap) == 1
    off_ap = gp.lower_ap_dma(offset_ap)
    assert len(off_ap) == 1
    in_ap.append(off_ap[0])
    ap_shape = in_.shape
    coef = 1
    for i in range(1, len(ap_shape)):
        coef *= ap_shape[i]
    in_ap[0].dynamic_ap_info = mybir.DynamicAccessPatternInfo(
        c=0,
        actual_ap=out.ap,
        indirect_dim_max_index=ap_shape[0],
        offset_expr=[
            mybir.DynamicAccessPatternOffsetExpr(
                coef=coef,
                aff_expr=mybir.DynamicAccessPatternOffsetExprAffExpr(
                    kind="IndirectArgId", arg_id=1),
            )
        ],
    )
    return gp.add_instruction(
        mybir.InstDMACopy(
            name=gp.bass.get_next_instruction_name(),
            queue=queue_name,
            mode="Copy",
            ins=in_ap,
            outs=out_ap,
            oob_is_err=True,
            cce_op=mybir.AluOpType.bypass,
        )
    )


def emit_kernel(tc, u_ap, idx_ap, t_ap, w_ap, amp_ap, b_ap, y_ap,
                b_loc=B_LOC, n_chunks=4, reps=1, gcols=GCOLS):
    """Emit the per-core program. All *_ap are DRAM APs:
      u_ap    [NMOD*nvec, R] f16   (gather table, mode-major)
      idx_ap  [128, 3*G] i32       idx[p, 3g+m] = b_i[g*128+p, m] + m*nvec
      t_ap    [3, b_loc] f16       row 0 = b_t_n; rows 1,2 = 1.0
      w_ap    [99, 256] f16        rows 0..96 W_freq/2pi; 97: 1536; 98: phi/2pi
      amp_ap  [128, 64] f16        col 32c+j = A[128c+p]; A=hypot(w1,w2)
      b_ap    [128, 1] f32         b_out replicated
      y_ap    [4, b_loc//4] f32    y_ap[q, 512*S+n] = y[2048*S + 512*q + n]
    """
    nc = tc.nc
    G = b_loc // P                  # groups of 128 batches
    GPB = 8                         # groups per compute block (1024 batches)
    assert G % GPB == 0
    NB = G // GPB
    assert NB % 2 == 0
    NS = NB // 2                    # superblocks of 2048 batches
    assert G % n_chunks == 0
    gpc = G // n_chunks             # groups per gather chunk
    assert gpc % GPB == 0
    bpc = gpc // GPB                # blocks per chunk

    from contextlib import ExitStack
    with ExitStack() as ctx:
        const_pool = ctx.enter_context(tc.tile_pool(name="const", bufs=1))
        idx_pool = ctx.enter_context(tc.tile_pool(name="idx", bufs=2))
        x_pool = ctx.enter_context(tc.tile_pool(name="xdata", bufs=1))
        xt_pool = ctx.enter_context(tc.tile_pool(name="xt", bufs=2))
        u_pool = ctx.enter_context(tc.tile_pool(name="uround", bufs=2))
        z_pool = ctx.enter_context(tc.tile_pool(name="zfrac", bufs=2))
        s_pool = ctx.enter_context(tc.tile_pool(name="sins", bufs=2))
        y_pool = ctx.enter_context(tc.tile_pool(name="yout", bufs=2))
        pt_pool = ctx.enter_context(tc.tile_pool(name="ptr", bufs=2, space="PSUM"))
        pr_pool = ctx.enter_context(tc.tile_pool(name="prj", bufs=1, space="PSUM"))
        py_pool = ctx.enter_context(tc.tile_pool(name="pyy", bufs=2, space="PSUM"))

        # constants to SBUF (loaded once; weights stay resident)
        w_sb = const_pool.tile([IN2, NFF], F16, name="w_sb")
        amp_sb = const_pool.tile([P, 64], F16, name="amp_sb")
        b_sb = const_pool.tile([P, 1], F32, name="b_sb")
        ident = const_pool.tile([P, P], F16, name="ident")

        zf32 = const_pool.tile([P, 1024], F32, name="zf32")
        zf16 = const_pool.tile([P, 512], F16, name="zf16")
        nc.vector.memset(zf32[:, :], 0.0)
        nc.vector.memset(zf16[:, :], 0.0)
        nc.sync.dma_start(out=w_sb[:, :], in_=w_ap)
        nc.sync.dma_start(out=amp_sb[:, :], in_=amp_ap)
        nc.sync.dma_start(out=b_sb[:, :], in_=b_ap)
        make_identity(nc, ident[:, :])

        # gather chunks: X_q [128, gpc*96] f16; X[p, 96 g + 32 m + e]
        #   = U[idx[p, 3g+m], e].  One indirect DMA covers `gcols` index
        #   columns (descriptor k pairs idx[p, j] with X[p, 32j:32j+32]).
        def do_gathers(idx_sb, rep):
            x_tiles = []
            n_inst = 0
            for q in range(n_chunks):
                xq = x_pool.tile([P, 96 * gpc], F16, tag=f"xq{q}",
                                 name=f"xq{q}_r{rep}")
                cols = 3 * gpc
                for j0 in range(0, cols, gcols):
                    j1 = min(j0 + gcols, cols)
                    jj0 = 3 * gpc * q + j0
                    jj1 = jj0 + (j1 - j0)
                    nc.gpsimd.indirect_dma_start(
                        out=xq[:, 32 * j0:32 * j1],
                        out_offset=None,
                        in_=u_ap,
                        in_offset=IndirectOffsetOnAxis(
                            ap=idx_sb[:, jj0:jj1], axis=0),
                    )
                x_tiles.append(xq)
            return x_tiles

        def do_block(x_tiles, rep, i, py, s_pos):
            """block i = 1024 batches; y accumulated into psum tile `py`
            at col-tile positions {2*s_pos, 2*s_pos+1}."""
            q = i // bpc
            xq = x_tiles[q]
            base_g = i * GPB - q * gpc

            xt = xt_pool.tile([P, 1024], F16, tag="xt",
                              name=f"xt{i}_r{rep}")
            # rows 96 (t) and 97/98 (ones) in one DMA, off the Pool queue
            nc.sync.dma_start(out=xt[96:99, :],
                              in_=t_ap[0:3, 1024 * i:1024 * (i + 1)])
            for h in range(2):
                pt = pt_pool.tile([P, 512], F16, tag="pt",
                                  name=f"pt{i}_{h}_r{rep}")
                for gl in range(4):
                    g_in = base_g + h * 4 + gl
                    nc.tensor.transpose(
                        out=pt[0:96, 128 * gl:128 * (gl + 1)],
                        in_=xq[:, 96 * g_in:96 * (g_in + 1)],
                        identity=ident[:, :])
                nc.vector.tensor_tensor(
                    out=xt[0:96, 512 * h:512 * (h + 1)],
                    in0=pt[0:96, 0:512], in1=zf16[0:96, 0:512],
                    op=mybir.AluOpType.add)

            sss = []
            for c in range(2):
                pc = pr_pool.tile([P, 1024], F32, tag=f"proj{c}",
                                  name=f"proj{c}_{i}_r{rep}")
                for h2 in range(2):
                    nc.tensor.matmul(
                        out=pc[:, 512 * h2:512 * (h2 + 1)],
                        lhsT=w_sb[0:IN2, 128 * c:128 * (c + 1)],
                        rhs=xt[0:IN2, 512 * h2:512 * (h2 + 1)],
                        start=True, stop=True)
                # u = round(pc): fp16 cast; pc sits in [1024, 2048) where
                # the fp16 grid is exactly the integers
                uc = u_pool.tile([P, 1024], F16, tag=f"u{c}",
                                 name=f"u{c}_{i}_r{rep}")
                nc.vector.tensor_tensor(
                    out=uc[:, :], in0=pc[:, :], in1=zf32[:, :],
                    op=mybir.AluOpType.add)
                # z = u - pc in [-0.5, 0.5]
                zc = z_pool.tile([P, 1024], F16, tag=f"z{c}",
                                 name=f"z{c}_{i}_r{rep}")
                nc.vector.tensor_tensor(
                    out=zc[:, :], in0=uc[:, :], in1=pc[:, :],
                    op=mybir.AluOpType.subtract)
                # s = sin(-2pi*z) = sin(proj + phi)
                sc = s_pool.tile([P, 1024], F16, tag=f"s{c}",
                                 name=f"s{c}_{i}_r{rep}")
                nc.scalar.activation(out=sc[:, :], in_=zc[:, :], func=SIN,
                                     bias=0.0, scale=-TWO_PI)
                sss.append(sc)

            for h2 in range(2):
                pos = 2 * s_pos + h2
                for c in range(2):
                    nc.tensor.matmul(
                        out=py[32 * pos:32 * pos + 32, :],
                        lhsT=amp_sb[0:P, 32 * c:32 * (c + 1)],
                        rhs=sss[c][:, 512 * h2:512 * (h2 + 1)],
                        start=(c == 0), stop=(c == 1),
                        tile_position=(0, 32 * pos))

        for rep in range(reps):
            # per-inference inputs: index array (re-)loaded every rep
            idx_sb = idx_pool.tile([P, 3 * G], I32, tag="idx",
                                   name=f"idx_sb_r{rep}")
            nc.sync.dma_start(out=idx_sb[:, :], in_=idx_ap)
            x_tiles = do_gathers(idx_sb, rep)
            for S in range(NS):
                py = py_pool.tile([P, 512], F32, tag="py",
                                  name=f"py{S}_r{rep}")
                do_block(x_tiles, rep, 2 * S, py, 0)
                do_block(x_tiles, rep, 2 * S + 1, py, 1)
                # full-width copy + b_out; DMA picks rows {0,32,64,96}
                ys = y_pool.tile([P, 512], F32, tag="ystage",
                                 name=f"ys{S}_r{rep}")
                nc.vector.tensor_scalar_add(
                    out=ys[:, :], in0=py[:, :], scalar1=b_sb[0:P, 0:1])
                nc.sync.dma_start(out=y_ap[0:4, 512 * S:512 * (S + 1)],
                                  in_=ys[0:128:32, 0:512])


def build_program(b_loc=B_LOC, nvec=NVEC, n_chunks=4, n_cores=N_CORES,
                  reps=1, gcols=GCOLS):
    """Build the full Bass module (one SPMD program for all cores)."""
    G = b_loc // P
    nc = bacc.Bacc("TRN2", target_bir_lowering=False, debug=False,
                   num_devices=n_cores, name="rff_embed",
                   num_swdge_queues=N_SWDGE_QUEUES)
    u_d = nc.dram_tensor("u_tab", [NMOD * nvec, R], F16, kind="ExternalInput").ap()
    idx_d = nc.dram_tensor("idx", [P, 3 * G], I32, kind="ExternalInput").ap()
    t_d = nc.dram_tensor("tvec", [3, b_loc], F16, kind="ExternalInput").ap()
    w_d = nc.dram_tensor("wfreq", [IN2, NFF], F16, kind="ExternalInput").ap()
    amp_d = nc.dram_tensor("amp", [P, 64], F16, kind="ExternalInput").ap()
    b_d = nc.dram_tensor("bout", [P, 1], F32, kind="ExternalInput").ap()
    y_d = nc.dram_tensor("y", [4, b_loc // 4], F32, kind="ExternalOutput").ap()

    with tile.TileContext(nc) as tc:
        emit_kernel(tc, u_d, idx_d, t_d, w_d, amp_d, b_d, y_d,
                    b_loc=b_loc, n_chunks=n_chunks, reps=reps, gcols=gcols)
    nc.compile()
    return nc


def prep_shared(U, W_freq, w_out, b_out, nvec=NVEC):
    """Host prep of the replicated (weight) tensors."""
    u_arr = np.ascontiguousarray(
        np.asarray(U).reshape(NMOD * nvec, R).astype(np.float16))
    inv = 1.0 / np.sqrt(np.float64(NFF))
    w_out = np.asarray(w_out)
    w1 = w_out[:NFF, 0].astype(np.float64) * inv
    w2 = w_out[NFF:, 0].astype(np.float64) * inv
    amp = np.hypot(w1, w2)
    phi = np.arctan2(w2, w1)
    w_arr = np.empty((IN2, NFF), np.float64)
    w_arr[:IN_DIM] = np.asarray(W_freq, np.float64) / (2 * np.pi)
    w_arr[IN_DIM] = POFF
    w_arr[IN_DIM + 1] = phi / (2 * np.pi)
    w_arr = np.ascontiguousarray(w_arr.astype(np.float16))
    amp_arr = np.empty((P, 64), np.float64)
    amp_arr[:, 0:32] = amp[:P, None]
    amp_arr[:, 32:64] = amp[P:, None]
    amp_arr = np.ascontiguousarray(amp_arr.astype(np.float16))
    b_arr = np.full((P, 1), np.asarray(b_out).reshape(()), np.float32)
    return u_arr, w_arr, amp_arr, b_arr


def prep_core(b_i, b_t, nvec=NVEC):
    """Host prep of one core's sharded index / t tensors."""
    b_loc = b_i.shape[0]
    G = b_loc // P
    offs = (np.arange(NMOD, dtype=np.int64) * nvec)
    idx = (np.asarray(b_i, np.int64).reshape(G, P, NMOD) + offs[None, None, :])
    idx = np.ascontiguousarray(
        idx.transpose(1, 0, 2).reshape(P, 3 * G).astype(np.int32))
    t1 = np.ones((3, b_loc), np.float16)
    t1[0] = np.asarray(b_t, np.float32).reshape(b_loc).astype(np.float16)
    return idx, t1


def unscramble_y(y_d, b_loc):
    """y_d [4, b_loc//4] -> y [b_loc] natural order."""
    ns = b_loc // 2048
    return np.ascontiguousarray(
        y_d.reshape(4, ns, 512).transpose(1, 0, 2).reshape(b_loc))


_PROGRAM_CACHE = {}


def kernel(b_i_n, b_t_n, U, W_freq, w_out, b_out):
    from concourse.bass_utils import run_bass_kernel_spmd

    key = "full"
    if key not in _PROGRAM_CACHE:
        _PROGRAM_CACHE[key] = build_program()
    nc = _PROGRAM_CACHE[key]

    u_arr, w_arr, amp_arr, b_arr = prep_shared(U, W_freq, w_out, b_out)
    in_maps = []
    for k in range(N_CORES):
        sl = slice(k * B_LOC, (k + 1) * B_LOC)
        idx, t1 = prep_core(np.asarray(b_i_n)[sl], np.asarray(b_t_n)[sl])
        in_maps.append({"u_tab": u_arr, "idx": idx, "tvec": t1,
                        "wfreq": w_arr, "amp": amp_arr, "bout": b_arr})

    res = run_bass_kernel_spmd(nc, in_maps, core_ids=list(range(N_CORES)))
    y = np.concatenate([unscramble_y(r["y"], B_LOC) for r in res.results])
    return y.reshape(B, 1).astype(np.float32)

